# revision 16
# baseline (speedup 1.0000x reference)
"""Ensemble low-bit-decoded 3x3 conv2d, data-parallel over 8 TRN2 NeuronCores.

Problem (hardcoded): x (16, 64, 160, 160) f32. 4 ensemble members; image b uses
ensemble n = b % 4. Weights (64, 64, 3, 3) per ensemble are decoded from the
tiny U/V/scale/biasq params:
    w = scale_n * (sigmoid(clip(U_n*V_0)) + 2*sigmoid(clip(U_n*V_1)) - biasq_n - 4)
then out[b] = conv2d(x[b], w_{b%4}, pad=1) + bias_{b%4}.
The decode is ~0.3 MFLOP of weight prep, done host-side in fp32/fp16 (same
rounding as the on-device path) while packing operands.

Sharding: core j gets images (2j, 2j+1); weights/bias replicated (tiny).

Kernel strategy per image:
  SBUF "parity" layout: padded image rows stored as pairs: partition p<64 =
  channel ci of one row parity, p>=64 = the other, at free column s*161 + col.
  A matmul with K=128 = (2 rows x 64 cin) and M=128 = (2 out rows x 64 cout)
  covers up to 4 conv taps at once; 6 matmuls (2 row-phases x 3 kw shifts)
  accumulate a PSUM tile of 2-3 output row-pairs, covering all 9 taps.
  Matmuls run PSUM-tile-major so each tile's bias-add (ACT) and store can
  start 6 matmuls after its inputs land.

DMA strategy: x and out live in DRAM in a parity-packed layout prepared on the
host (free): xp[i, par*64+c, s, :] with par0 = odd rows shifted (slot s -> row
2s-1, slot 0 = zero pad row) and par1 = even rows (slot s -> row 2s, slot 80 =
zero pad row). Each band load/store is a 128-partition DMA whose per-partition
region is fully contiguous (multi-KB descriptors instead of 640 B), keeping
the 16 SDMA engines at HBM line rate. Loads ride the SP HWDGE ring, stores the
ACT ring; each band is split in thirds so downstream work starts early
(subtile deps).
"""

import os

import numpy as np

import concourse.bass as bass
import concourse.mybir as mybir
import concourse.tile as tile
from concourse import bacc

N = 4
CIN = 64
COUT = 64
KS = 3
NB = 2  # weight bits
H = 160
W = 160
N_CORES = 8
N_IMG = 2  # images per core

F32 = mybir.dt.float32


def build_nc(
    n_img=N_IMG,
    h=H,
    w=W,
    band_out_pairs=20,
    st_pairs=3,
    mm_dtype=mybir.dt.float16,
):
    """Build the single-core Bass program (SPMD: all cores run this)."""
    wr = w + 1  # row-pair pitch in the band tile (shared pad col)
    out_pairs = h // 2  # 80
    n_slots = out_pairs + 1  # 81 pair-slots in the packed x (incl. pad rows)
    assert out_pairs % band_out_pairs == 0
    n_bands = out_pairs // band_out_pairs
    npb = band_out_pairs + 1  # input pair-slots needed per band
    ld3 = npb // 3  # 3-way load split (7 slots each)
    assert ld3 * 3 == npb

    nc = bacc.Bacc("TRN2", target_bir_lowering=False, num_swdge_queues=4)

    xp = nc.dram_tensor(
        "xp", (n_img, 128, n_slots, w), mm_dtype, kind="ExternalInput"
    )
    lwd = nc.dram_tensor(
        "lwd", (n_img, 128, 6 * 2 * 64), mm_dtype, kind="ExternalInput"
    )
    bsd = nc.dram_tensor("bsd", (128, n_img), F32, kind="ExternalInput")
    outp = nc.dram_tensor(
        "outp", (n_img, 128, out_pairs, w), F32, kind="ExternalOutput"
    )

    AF = mybir.ActivationFunctionType

    with tile.TileContext(nc) as tc:
        with (
            tc.tile_pool(name="params", bufs=1) as ppool,
            tc.tile_pool(name="band", bufs=3) as bpool,
            tc.tile_pool(name="stage", bufs=3) as spool,
            tc.tile_pool(name="obuf", bufs=3) as opool,
            tc.tile_pool(name="psum", bufs=7, space="PSUM") as pspool,
            tc.tile_pool(name="warmpsum", bufs=1, space="PSUM") as wpspool,
        ):
            # ---- PE clock pre-warm: the HAM gate holds the PE at 1.2 GHz
            # until it sees ~3.4us of sustained activity; burn that window on
            # dummy matmuls while the first loads are still in flight.
            warm = ppool.tile([128, 512], mm_dtype, tag="warm")
            nc.gpsimd.memset(warm[:], 0.0)
            wps = wpspool.tile([128, 512], F32, tag="warmps", name="warmps")
            for _ in range(7):
                nc.tensor.matmul(
                    wps[:], warm[:, 0:128], warm[:], start=True, stop=True
                )

            # ---- pre-decoded stacked lhsT weight tiles + output bias
            # lw[:, widx, j, co]: widx 0..2 = phase1 kw, 3..5 = phase2 kw
            # These tiny DMAs ride the ACT ring (idle until stores begin) so
            # the SP ring is pure input loads in consumption order.
            lhs = []
            for i in range(n_img):
                lw = ppool.tile([128, 6, 2, 64], mm_dtype, tag=f"lw{i}")
                nc.scalar.dma_start(out=lw[:], in_=lwd[i])
                lhs.append(lw)
            b_sb = ppool.tile([128, n_img], F32, tag="bias")
            nc.scalar.dma_start(out=b_sb[:], in_=bsd[:, :])

            # super-tile split of each band (out-pairs per PSUM tile)
            sts = []
            rem = band_out_pairs
            while rem > 0:
                k = min(st_pairs, rem)
                sts.append(k)
                rem -= k
            offs = [sum(sts[:j]) for j in range(len(sts))]
            # store split points: after these tiles, flush ob rows so far
            flush_after = {
                2: (0, offs[3]),
                4: (offs[3], offs[5]),
                5: (offs[5], offs[6]),
            }
            last_flush = offs[6]

            for i in range(n_img):
                prev_b3 = None
                for band in range(n_bands):
                    s0p = band * band_out_pairs  # first pair-slot == first out pair
                    # shared-pad layout: pair-slot t's data at cols t*(w+1)+1..+w;
                    # col t*(w+1) is both row t's left pad and row t-1's right
                    # pad, so the matmul moving operand is 1D-contiguous.
                    bt = bpool.tile([128, npb * wr + 1], mm_dtype, tag="band")
                    b3 = bt[:, 0 : npb * wr].rearrange("p (t c) -> p t c", t=npb)
                    # contiguous 128-partition loads (fp32) in thirds so the
                    # cast + first matmuls start at ~0.6 MB granularity.
                    # Interior bands reuse the previous band's last pair-slot
                    # from SBUF instead of re-loading it from HBM.
                    stg = spool.tile([128, npb, w], mm_dtype, tag="stg")
                    lo0 = 0 if band == 0 else 1
                    ranges = [(lo0, lo0 + ld3), (lo0 + ld3, lo0 + 2 * ld3),
                              (lo0 + 2 * ld3, npb)]
                    for lo, hi in ranges:
                        nc.sync.dma_start(
                            out=stg[:, lo:hi, :],
                            in_=xp[i, :, s0p + lo : s0p + hi, :],
                        )
                    # zero the shared pad cols (every wr-th col); the virtual
                    # edge rows are pre-zeroed in the packed DRAM layout
                    nc.gpsimd.memset(bt[:, 0 : npb * wr + 1 : wr], 0.0)
                    if band > 0:
                        nc.vector.tensor_copy(
                            b3[:, 0, 1 : w + 1],
                            prev_b3[:, band_out_pairs, 1 : w + 1],
                        )
                    for lo, hi in ranges:
                        nc.vector.tensor_copy(
                            b3[:, lo:hi, 1 : w + 1], stg[:, lo:hi, :]
                        )
                    prev_b3 = b3

                    ob = opool.tile([128, band_out_pairs, w], F32, tag="ob")
                    bias_ap = b_sb[:, i : i + 1]
                    for sti, k in enumerate(sts):
                        ps = pspool.tile([128, k * wr], F32, tag="ps", name="ps")
                        o = offs[sti]
                        f = k * wr - 1
                        for widx in range(6):
                            kw = widx % 3
                            base = (o + widx // 3) * wr
                            nc.tensor.matmul(
                                ps[:, 0:f],
                                lhs[i][:, widx, :, :],
                                bt[:, base + kw : base + kw + f],
                                start=(widx == 0),
                                stop=(widx == 5),
                            )
                        ps3 = ps.rearrange("p (t c) -> p t c", t=k)
                        nc.scalar.activation(
                            ob[:, o : o + k, :],
                            ps3[:, :, 0:w],
                            AF.Identity,
                            bias=bias_ap,
                            scale=1.0,
                        )
                        if sti in flush_after:
                            lo, hi = flush_after[sti]
                            nc.scalar.dma_start(
                                out=outp[i, :, s0p + lo : s0p + hi, :],
                                in_=ob[:, lo:hi, :],
                            )
                    nc.scalar.dma_start(
                        out=outp[i, :, s0p + last_flush : s0p + band_out_pairs, :],
                        in_=ob[:, last_flush:band_out_pairs, :],
                    )

    nc.compile()
    return nc


_NC_CACHE = {}


def _get_nc():
    if "nc" not in _NC_CACHE:
        _NC_CACHE["nc"] = build_nc()
    return _NC_CACHE["nc"]


def _decode_weights(U, V, scale, biasq, bias):
    """Host-side weight decode + lhsT packing (per ensemble).

    Returns lw (N, 128, 6, 2, 64) fp16 and bias bstack (N, 128) f32.
    lw partition p<64 = ci, p>=64 = ci (other row parity); widx = phase*3+kw.
    """
    theta = U[:, :, 0][:, None, :] * V[:, :, 0][None, :, :]  # (N, NB, D)
    soft = 1.0 / (1.0 + np.exp(-np.clip(theta, -10.0, 10.0)))
    integer = soft[:, 0, :] + 2.0 * soft[:, 1, :]  # (N, D)
    wv = scale * (integer - biasq - 2.0**NB)  # (N, D)
    # D is (co, ci, kh, kw) -> (n, ci, kh, kw, co)
    w5 = wv.reshape(N, COUT, CIN, KS, KS).transpose(0, 2, 3, 4, 1)
    w5 = np.ascontiguousarray(w5).astype(np.float16)
    lw = np.zeros((N, 128, 6, 2, COUT), np.float16)
    for kw in range(KS):
        # phase 1 (rhs slots m, m+1 -> out rows 2m, 2m+1):
        #   q0 j0: kh0;  q1 j0: kh1, j1: kh0
        lw[:, 0:64, kw, 0, :] = w5[:, :, 0, kw, :]
        lw[:, 64:128, kw, 0, :] = w5[:, :, 1, kw, :]
        lw[:, 64:128, kw, 1, :] = w5[:, :, 0, kw, :]
        # phase 2: q0 j0: kh2, j1: kh1;  q1 j1: kh2
        lw[:, 0:64, 3 + kw, 0, :] = w5[:, :, 2, kw, :]
        lw[:, 0:64, 3 + kw, 1, :] = w5[:, :, 1, kw, :]
        lw[:, 64:128, 3 + kw, 1, :] = w5[:, :, 2, kw, :]
    bn = bias.reshape(N, COUT)
    bstack = np.concatenate([bn, bn], axis=1).astype(np.float32)  # (N, 128)
    return lw, bstack


def _pack_x(x):
    """Parity-pack x (16, 64, H, W) -> (16, 128, 81, W) with pad rows baked in.

    Partition par*64+c, pair-slot s:
      par0: real row 2s-1 (slot 0 = zero = virtual top pad row)
      par1: real row 2s   (slot 80 = zero = virtual bottom pad row)
    """
    B = x.shape[0]
    n_slots = H // 2 + 1
    xp = np.zeros((B, 2, CIN, n_slots, W), np.float16)
    xp[:, 0, :, 1:] = x[:, :, 1::2, :]
    xp[:, 1, :, :-1] = x[:, :, 0::2, :]
    return xp.reshape(B, 2 * CIN, n_slots, W)


LAST_RESULT = None


def _ensure_ntff_hook():
    """The container's antenv package lacks axon_hooks; synthesize it so
    run_bass_kernel_spmd(trace=True) can register the NTFF profiler."""
    import sys
    import types

    if "antenv.axon_hooks" in sys.modules:
        return True
    try:
        import antenv
        from trn_agent_boot.trn_boot import _ntff_profile_via_ctypes

        hook = _ntff_profile_via_ctypes("/opt/axon/libaxon_pjrt.so")
        mod = types.ModuleType("antenv.axon_hooks")
        mod._hook = hook
        mod.get_axon_ntff_profile_hook = lambda: mod._hook
        mod.set_axon_ntff_profile_hook = lambda h: setattr(mod, "_hook", h)
        sys.modules["antenv.axon_hooks"] = mod
        antenv.axon_hooks = mod
        return hook is not None
    except Exception as e:  # degrade to untraced run
        print(f"ntff hook setup failed: {type(e).__name__}: {e}")
        return False


def kernel(x, U, V, twopow, scale, biasq, bias):
    from concourse.bass_utils import run_bass_kernel_spmd

    global LAST_RESULT
    x = np.asarray(x, np.float32)
    lw, bstack = _decode_weights(
        np.asarray(U, np.float64),
        np.asarray(V, np.float64),
        np.asarray(scale, np.float64),
        np.asarray(biasq, np.float64),
        np.asarray(bias, np.float32),
    )
    xp = _pack_x(x)

    in_maps = []
    for j in range(N_CORES):
        bs = [N_IMG * j + t for t in range(N_IMG)]
        ns = [b % N for b in bs]
        in_maps.append(
            {
                "xp": np.ascontiguousarray(xp[bs]),
                "lwd": np.ascontiguousarray(lw[ns]).reshape(N_IMG, 128, -1),
                "bsd": np.ascontiguousarray(bstack[ns].T),
            }
        )

    nc = _get_nc()
    trace = bool(os.environ.get("KERNEL_TRACE"))
    if trace:
        trace = _ensure_ntff_hook()
    tmpdir = os.environ.get("KERNEL_TRACE_DIR") or None
    res = run_bass_kernel_spmd(
        nc, in_maps, list(range(N_CORES)), trace=trace, tmpdir=tmpdir
    )
    LAST_RESULT = res

    out = np.empty((16, COUT, H, W), np.float32)
    for j in range(N_CORES):
        op = res.results[j]["outp"].reshape(N_IMG, 2, COUT, H // 2, W)
        out[N_IMG * j : N_IMG * (j + 1), :, 0::2, :] = op[:, 0]
        out[N_IMG * j : N_IMG * (j + 1), :, 1::2, :] = op[:, 1]
    return out


# revision 19
# speedup vs baseline: 1.0440x; 1.0440x over previous
"""Ensemble low-bit-decoded 3x3 conv2d, data-parallel over 8 TRN2 NeuronCores.

Problem (hardcoded): x (16, 64, 160, 160) f32. 4 ensemble members; image b uses
ensemble n = b % 4. Weights (64, 64, 3, 3) per ensemble are decoded from the
tiny U/V/scale/biasq params:
    w = scale_n * (sigmoid(clip(U_n*V_0)) + 2*sigmoid(clip(U_n*V_1)) - biasq_n - 4)
then out[b] = conv2d(x[b], w_{b%4}, pad=1) + bias_{b%4}.
The decode is ~0.3 MFLOP of weight prep, done host-side in fp32/fp16 (same
rounding as the on-device path) while packing operands.

Sharding: core j gets images (2j, 2j+1); weights/bias replicated (tiny).

Kernel strategy per image:
  SBUF "parity" layout: padded image rows stored as pairs: partition p<64 =
  channel ci of one row parity, p>=64 = the other, at free column s*161 + col.
  A matmul with K=128 = (2 rows x 64 cin) and M=128 = (2 out rows x 64 cout)
  covers up to 4 conv taps at once; 6 matmuls (2 row-phases x 3 kw shifts)
  accumulate a PSUM tile of 2-3 output row-pairs, covering all 9 taps.
  Matmuls run PSUM-tile-major so each tile's bias-add (ACT) and store can
  start 6 matmuls after its inputs land.

DMA strategy: x and out live in DRAM in a parity-packed layout prepared on the
host (free): xp[i, par*64+c, s, :] with par0 = odd rows shifted (slot s -> row
2s-1, slot 0 = zero pad row) and par1 = even rows (slot s -> row 2s, slot 80 =
zero pad row). Each band load/store is a 128-partition DMA whose per-partition
region is fully contiguous (multi-KB descriptors instead of 640 B), keeping
the 16 SDMA engines at HBM line rate. Loads ride the SP HWDGE ring, stores the
ACT ring; each band is split in thirds so downstream work starts early
(subtile deps).
"""

import os

import numpy as np

import concourse.bass as bass
import concourse.mybir as mybir
import concourse.tile as tile
from concourse import bacc

N = 4
CIN = 64
COUT = 64
KS = 3
NB = 2  # weight bits
H = 160
W = 160
N_CORES = 8
N_IMG = 2  # images per core

F32 = mybir.dt.float32


def build_nc(
    n_img=N_IMG,
    h=H,
    w=W,
    band_out_pairs=20,
    st_pairs=3,
    mm_dtype=mybir.dt.float16,
):
    """Build the single-core Bass program (SPMD: all cores run this)."""
    wr = w + 1  # row-pair pitch in the band tile (shared pad col)
    out_pairs = h // 2  # 80
    n_slots = out_pairs + 1  # 81 pair-slots in the packed x (incl. pad rows)
    assert out_pairs % band_out_pairs == 0
    n_bands = out_pairs // band_out_pairs
    npb = band_out_pairs + 1  # input pair-slots needed per band
    ld3 = npb // 3  # 3-way load split (7 slots each)
    assert ld3 * 3 == npb

    nc = bacc.Bacc("TRN2", target_bir_lowering=False, num_swdge_queues=4)

    xp = nc.dram_tensor(
        "xp", (n_img, 128, n_slots, w), mm_dtype, kind="ExternalInput"
    )
    lwd = nc.dram_tensor(
        "lwd", (n_img, 128, 6 * 2 * 64), mm_dtype, kind="ExternalInput"
    )
    bsd = nc.dram_tensor("bsd", (128, n_img), F32, kind="ExternalInput")
    outp = nc.dram_tensor(
        "outp", (n_img, 128, out_pairs, w), F32, kind="ExternalOutput"
    )

    AF = mybir.ActivationFunctionType

    with tile.TileContext(nc) as tc:
        with (
            tc.tile_pool(name="params", bufs=1) as ppool,
            tc.tile_pool(name="band", bufs=3) as bpool,
            tc.tile_pool(name="stage", bufs=3) as spool,
            tc.tile_pool(name="obuf", bufs=3) as opool,
            tc.tile_pool(name="psum", bufs=7, space="PSUM") as pspool,
            tc.tile_pool(name="warmpsum", bufs=1, space="PSUM") as wpspool,
        ):
            # ---- PE clock pre-warm: the HAM gate holds the PE at 1.2 GHz
            # until it sees ~3.4us of sustained activity; burn that window on
            # dummy matmuls while the first loads are still in flight.
            warm = ppool.tile([128, 512], mm_dtype, tag="warm")
            nc.vector.memset(warm[:], 0.0)
            wps = wpspool.tile([128, 512], F32, tag="warmps", name="warmps")
            for _ in range(7):
                nc.tensor.matmul(
                    wps[:], warm[:, 0:128], warm[:], start=True, stop=True
                )

            # ---- pre-decoded stacked lhsT weight tiles + output bias
            # lw[:, widx, j, co]: widx 0..2 = phase1 kw, 3..5 = phase2 kw
            # These tiny DMAs ride the ACT ring (idle until stores begin) so
            # the SP ring is pure input loads in consumption order.
            lhs = []
            for i in range(n_img):
                lw = ppool.tile([128, 6, 2, 64], mm_dtype, tag=f"lw{i}")
                nc.scalar.dma_start(out=lw[:], in_=lwd[i])
                lhs.append(lw)
            b_sb = ppool.tile([128, n_img], F32, tag="bias")
            nc.scalar.dma_start(out=b_sb[:], in_=bsd[:, :])

            # super-tile split of each band (out-pairs per PSUM tile)
            sts = []
            rem = band_out_pairs
            while rem > 0:
                k = min(st_pairs, rem)
                sts.append(k)
                rem -= k
            offs = [sum(sts[:j]) for j in range(len(sts))]
            # store split points: after these tiles, flush ob rows so far
            flush_after = {
                2: (0, offs[3]),
                4: (offs[3], offs[5]),
                5: (offs[5], offs[6]),
            }
            last_flush = offs[6]

            for i in range(n_img):
                prev_b3 = None
                for band in range(n_bands):
                    s0p = band * band_out_pairs  # first pair-slot == first out pair
                    # shared-pad layout: pair-slot t's data at cols t*(w+1)+1..+w;
                    # col t*(w+1) is both row t's left pad and row t-1's right
                    # pad, so the matmul moving operand is 1D-contiguous.
                    bt = bpool.tile([128, npb * wr + 1], mm_dtype, tag="band")
                    b3 = bt[:, 0 : npb * wr].rearrange("p (t c) -> p t c", t=npb)
                    # contiguous 128-partition loads (fp32) in thirds so the
                    # cast + first matmuls start at ~0.6 MB granularity.
                    # Interior bands reuse the previous band's last pair-slot
                    # from SBUF instead of re-loading it from HBM.
                    stg = spool.tile([128, npb, w], mm_dtype, tag="stg")
                    lo0 = 0 if band == 0 else 1
                    ranges = [(lo0, lo0 + ld3), (lo0 + ld3, lo0 + 2 * ld3),
                              (lo0 + 2 * ld3, npb)]
                    for lo, hi in ranges:
                        nc.sync.dma_start(
                            out=stg[:, lo:hi, :],
                            in_=xp[i, :, s0p + lo : s0p + hi, :],
                        )
                    # zero the shared pad cols (every wr-th col); the virtual
                    # edge rows are pre-zeroed in the packed DRAM layout
                    nc.gpsimd.memset(bt[:, 0 : npb * wr + 1 : wr], 0.0)
                    if band > 0:
                        nc.vector.tensor_copy(
                            b3[:, 0, 1 : w + 1],
                            prev_b3[:, band_out_pairs, 1 : w + 1],
                        )
                    for lo, hi in ranges:
                        nc.vector.tensor_copy(
                            b3[:, lo:hi, 1 : w + 1], stg[:, lo:hi, :]
                        )
                    prev_b3 = b3

                    ob = opool.tile([128, band_out_pairs, w], F32, tag="ob")
                    bias_ap = b_sb[:, i : i + 1]
                    # in the final bands all loads are done, so stores ride
                    # the idle SP ring instead of queueing descriptor-gen on
                    # the ACT sequencer between bias-add ops
                    last2 = i == n_img - 1 and band >= n_bands - 2
                    string = nc.sync if last2 else nc.scalar
                    for sti, k in enumerate(sts):
                        ps = pspool.tile([128, k * wr], F32, tag="ps", name="ps")
                        o = offs[sti]
                        f = k * wr - 1
                        for widx in range(6):
                            kw = widx % 3
                            base = (o + widx // 3) * wr
                            nc.tensor.matmul(
                                ps[:, 0:f],
                                lhs[i][:, widx, :, :],
                                bt[:, base + kw : base + kw + f],
                                start=(widx == 0),
                                stop=(widx == 5),
                            )
                        ps3 = ps.rearrange("p (t c) -> p t c", t=k)
                        nc.scalar.activation(
                            ob[:, o : o + k, :],
                            ps3[:, :, 0:w],
                            AF.Identity,
                            bias=bias_ap,
                            scale=1.0,
                        )
                        if sti in flush_after:
                            lo, hi = flush_after[sti]
                            string.dma_start(
                                out=outp[i, :, s0p + lo : s0p + hi, :],
                                in_=ob[:, lo:hi, :],
                            )
                    string.dma_start(
                        out=outp[i, :, s0p + last_flush : s0p + band_out_pairs, :],
                        in_=ob[:, last_flush:band_out_pairs, :],
                    )

    nc.compile()
    return nc


_NC_CACHE = {}


def _get_nc():
    if "nc" not in _NC_CACHE:
        _NC_CACHE["nc"] = build_nc()
    return _NC_CACHE["nc"]


def _decode_weights(U, V, scale, biasq, bias):
    """Host-side weight decode + lhsT packing (per ensemble).

    Returns lw (N, 128, 6, 2, 64) fp16 and bias bstack (N, 128) f32.
    lw partition p<64 = ci, p>=64 = ci (other row parity); widx = phase*3+kw.
    """
    theta = U[:, :, 0][:, None, :] * V[:, :, 0][None, :, :]  # (N, NB, D)
    soft = 1.0 / (1.0 + np.exp(-np.clip(theta, -10.0, 10.0)))
    integer = soft[:, 0, :] + 2.0 * soft[:, 1, :]  # (N, D)
    wv = scale * (integer - biasq - 2.0**NB)  # (N, D)
    # D is (co, ci, kh, kw) -> (n, ci, kh, kw, co)
    w5 = wv.reshape(N, COUT, CIN, KS, KS).transpose(0, 2, 3, 4, 1)
    w5 = np.ascontiguousarray(w5).astype(np.float16)
    lw = np.zeros((N, 128, 6, 2, COUT), np.float16)
    for kw in range(KS):
        # phase 1 (rhs slots m, m+1 -> out rows 2m, 2m+1):
        #   q0 j0: kh0;  q1 j0: kh1, j1: kh0
        lw[:, 0:64, kw, 0, :] = w5[:, :, 0, kw, :]
        lw[:, 64:128, kw, 0, :] = w5[:, :, 1, kw, :]
        lw[:, 64:128, kw, 1, :] = w5[:, :, 0, kw, :]
        # phase 2: q0 j0: kh2, j1: kh1;  q1 j1: kh2
        lw[:, 0:64, 3 + kw, 0, :] = w5[:, :, 2, kw, :]
        lw[:, 0:64, 3 + kw, 1, :] = w5[:, :, 1, kw, :]
        lw[:, 64:128, 3 + kw, 1, :] = w5[:, :, 2, kw, :]
    bn = bias.reshape(N, COUT)
    bstack = np.concatenate([bn, bn], axis=1).astype(np.float32)  # (N, 128)
    return lw, bstack


def _pack_x(x):
    """Parity-pack x (16, 64, H, W) -> (16, 128, 81, W) with pad rows baked in.

    Partition par*64+c, pair-slot s:
      par0: real row 2s-1 (slot 0 = zero = virtual top pad row)
      par1: real row 2s   (slot 80 = zero = virtual bottom pad row)
    """
    B = x.shape[0]
    n_slots = H // 2 + 1
    xp = np.zeros((B, 2, CIN, n_slots, W), np.float16)
    xp[:, 0, :, 1:] = x[:, :, 1::2, :]
    xp[:, 1, :, :-1] = x[:, :, 0::2, :]
    return xp.reshape(B, 2 * CIN, n_slots, W)


LAST_RESULT = None


def _ensure_ntff_hook():
    """The container's antenv package lacks axon_hooks; synthesize it so
    run_bass_kernel_spmd(trace=True) can register the NTFF profiler."""
    import sys
    import types

    if "antenv.axon_hooks" in sys.modules:
        return True
    try:
        import antenv
        from trn_agent_boot.trn_boot import _ntff_profile_via_ctypes

        hook = _ntff_profile_via_ctypes("/opt/axon/libaxon_pjrt.so")
        mod = types.ModuleType("antenv.axon_hooks")
        mod._hook = hook
        mod.get_axon_ntff_profile_hook = lambda: mod._hook
        mod.set_axon_ntff_profile_hook = lambda h: setattr(mod, "_hook", h)
        sys.modules["antenv.axon_hooks"] = mod
        antenv.axon_hooks = mod
        return hook is not None
    except Exception as e:  # degrade to untraced run
        print(f"ntff hook setup failed: {type(e).__name__}: {e}")
        return False


def kernel(x, U, V, twopow, scale, biasq, bias):
    from concourse.bass_utils import run_bass_kernel_spmd

    global LAST_RESULT
    x = np.asarray(x, np.float32)
    lw, bstack = _decode_weights(
        np.asarray(U, np.float64),
        np.asarray(V, np.float64),
        np.asarray(scale, np.float64),
        np.asarray(biasq, np.float64),
        np.asarray(bias, np.float32),
    )
    xp = _pack_x(x)

    in_maps = []
    for j in range(N_CORES):
        bs = [N_IMG * j + t for t in range(N_IMG)]
        ns = [b % N for b in bs]
        in_maps.append(
            {
                "xp": np.ascontiguousarray(xp[bs]),
                "lwd": np.ascontiguousarray(lw[ns]).reshape(N_IMG, 128, -1),
                "bsd": np.ascontiguousarray(bstack[ns].T),
            }
        )

    nc = _get_nc()
    trace = bool(os.environ.get("KERNEL_TRACE"))
    if trace:
        trace = _ensure_ntff_hook()
    tmpdir = os.environ.get("KERNEL_TRACE_DIR") or None
    res = run_bass_kernel_spmd(
        nc, in_maps, list(range(N_CORES)), trace=trace, tmpdir=tmpdir
    )
    LAST_RESULT = res

    out = np.empty((16, COUT, H, W), np.float32)
    for j in range(N_CORES):
        op = res.results[j]["outp"].reshape(N_IMG, 2, COUT, H // 2, W)
        out[N_IMG * j : N_IMG * (j + 1), :, 0::2, :] = op[:, 0]
        out[N_IMG * j : N_IMG * (j + 1), :, 1::2, :] = op[:, 1]
    return out
